# revision 1
# baseline (speedup 1.0000x reference)
"""Trainium2 Bass kernel for ControlLoRACrossAttnProcessor.

Head-parallel sharding over 8 NeuronCores: core c owns attention head c
(columns c*128:(c+1)*128 of Wq/Wk/Wv, rows of the same range in Wo's
contraction dim). Each core computes its head's full attention and a
partial output projection; the host sums the 8 partials. The rank-4
LoRA path is sharded by sequence rows (core c owns rows c*512:(c+1)*512)
and returned as a separate small output that the host adds in, together
with the output bias (added exactly once per row).

All matmuls run as float32r (TF32-like PE mode, full rate at moving
dim >= 256) with fp32 accumulation in PSUM. Attention uses the
transposed-scores layout: scores^T tiles [k=128, q=512] so softmax's
exp rides ScalarE and the k-sums ride TensorE (ones-vector matmul);
normalization is deferred to the output projection (divide commutes
with the linear Wo projection).
"""

import sys
import types

for _p in ("/opt/trn_rl_repo", "/root/.axon_site"):
    if _p not in sys.path:
        sys.path.insert(0, _p)

import numpy as np

import concourse.bass as bass  # noqa: E402
import concourse.mybir as mybir  # noqa: E402
from concourse import bacc  # noqa: E402
from concourse.bass_utils import run_bass_kernel_spmd  # noqa: E402
from concourse.tile import TileContext  # noqa: E402
from concourse.masks import make_identity  # noqa: E402

dt = mybir.dt

B, S, D = 2, 2048, 1024
H = 8
HD = 128
RANK = 4
N_CORES = 8
SG = B * S            # 4096 flattened rows
ROWS_PER_CORE = SG // N_CORES  # 512
NSTRIP = S // 512     # 4 query strips of 512 per batch
NKT = S // 128        # 16 key tiles of 128 per batch
NQT = 512 // 128      # 4 query tiles per strip
INV_SQRT_HD = 1.0 / np.sqrt(np.float32(HD))

F32 = dt.float32
F32R = dt.float32r

_CACHE = {}


def build_program():
    if "nc" in _CACHE:
        return _CACHE["nc"]

    nc = bacc.Bacc("TRN2", target_bir_lowering=False, debug=False,
                   num_devices=N_CORES)

    xT = nc.declare_dram_parameter("xT", [D, SG], F32R, isOutput=False)
    wqT = nc.declare_dram_parameter("wqT", [D, HD], F32R, isOutput=False)
    wkT = nc.declare_dram_parameter("wkT", [D, HD], F32R, isOutput=False)
    wvT = nc.declare_dram_parameter("wvT", [D, HD], F32R, isOutput=False)
    woT = nc.declare_dram_parameter("woT", [HD, D], F32R, isOutput=False)
    cT = nc.declare_dram_parameter("cT", [D, ROWS_PER_CORE], F32R, isOutput=False)
    ldT = nc.declare_dram_parameter("ldT", [D, RANK], F32R, isOutput=False)
    luT = nc.declare_dram_parameter("luT", [RANK, D], F32R, isOutput=False)
    bo = nc.declare_dram_parameter("bo", [1, D], F32, isOutput=False)
    out = nc.declare_dram_parameter("out", [SG, D], F32, isOutput=True)
    lora_out = nc.declare_dram_parameter("lora_out", [ROWS_PER_CORE, D], F32,
                                         isOutput=True)

    with TileContext(nc) as tc:
        with tc.tile_pool(name="const", bufs=1) as constp, \
             tc.tile_pool(name="wts", bufs=1) as wts, \
             tc.tile_pool(name="op_ps", bufs=2, space="PSUM") as op_ps, \
             tc.tile_pool(name="sc_ps", bufs=2, space="PSUM") as sc_ps, \
             tc.tile_pool(name="at_ps", bufs=1, space="PSUM") as at_ps, \
             tc.tile_pool(name="sum_ps", bufs=1, space="PSUM") as sum_ps, \
             tc.tile_pool(name="xt", bufs=2) as xtp, \
             tc.tile_pool(name="qkv", bufs=2) as qkvp, \
             tc.tile_pool(name="es", bufs=5) as esp, \
             tc.tile_pool(name="small", bufs=2) as smallp, \
             tc.tile_pool(name="outp", bufs=2) as outp, \
             tc.tile_pool(name="ct", bufs=1) as ctp:

            # ---- constants & weight loads (smallest first) ----
            ident = constp.tile([128, 128], F32, tag="ident")
            make_identity(nc, ident[:])
            ones_f = constp.tile([128, 1], F32, tag="onesf")
            nc.vector.memset(ones_f[:], 1.0)
            ones = constp.tile([128, 1], F32R, tag="ones")
            nc.vector.tensor_copy(ones[:], ones_f[:])

            wq_sb = wts.tile([128, D], F32R, tag="wq")
            wk_sb = wts.tile([128, D], F32R, tag="wk")
            wv_sb = wts.tile([128, D], F32R, tag="wv")
            wo_sb = wts.tile([HD, D], F32R, tag="wo")
            lu_sb = wts.tile([RANK, D], F32R, tag="lu")
            ld_sb = wts.tile([128, 8 * RANK], F32R, tag="ld")
            bo_sb = wts.tile([1, D], F32, tag="bo")
            bo_bc = wts.tile([128, D], F32, tag="bobc")
            for _w_sb, _wT in ((wq_sb, wqT), (wk_sb, wkT), (wv_sb, wvT)):
                nc.sync.dma_start(
                    out=_w_sb[:].rearrange("p (t m) -> p t m", t=8),
                    in_=_wT[:].rearrange("(t p) m -> p t m", p=128))

            xt_tiles = {}

            def load_xt(b):
                tiles = [xtp.tile([128, 4 * S], F32R, tag="xt",
                                  name=f"xt{b}_{h}") for h in range(2)]
                for h in range(2):
                    nc.sync.dma_start(
                        out=tiles[h][:].rearrange("p (t s) -> p t s", t=4),
                        in_=xT[h * 512:(h + 1) * 512, b * S:(b + 1) * S]
                        .rearrange("(t p) s -> p t s", p=128))
                xt_tiles[b] = tiles

            load_xt(0)

            ct_sb = ctp.tile([128, 8 * ROWS_PER_CORE], F32R, tag="ct")
            nc.sync.dma_start(
                out=ct_sb[:].rearrange("p (t m) -> p t m", t=8),
                in_=cT[:].rearrange("(t p) m -> p t m", p=128))
            nc.sync.dma_start(out=wo_sb[:], in_=woT[:])
            nc.sync.dma_start(out=lu_sb[:], in_=luT[:])
            nc.sync.dma_start(out=ld_sb[:].rearrange("p (t m) -> p t m", t=8),
                              in_=ldT[:].rearrange("(t p) m -> p t m", p=128))
            nc.sync.dma_start(out=bo_sb[:], in_=bo[:])
            nc.gpsimd.partition_broadcast(bo_bc[:], bo_sb[:])

            # PE warmup while the first activation DMAs land: keeps the HAM
            # clock-gate warm and fills the otherwise-idle load window.
            wu_ps = sum_ps.tile([1, 512], F32, tag="sums")
            for _wu in range(48):
                nc.tensor.matmul(wu_ps[:], ones[:], wq_sb[:, 0:512],
                                 start=True, stop=True)

            def emit_lora():
                # rows [c*512, (c+1)*512) of up(down(ctrl)) + bias
                dn_ps = op_ps.tile([RANK, ROWS_PER_CORE], F32, tag="op",
                                   name="dn_ps")
                for d in range(8):
                    nc.tensor.matmul(
                        dn_ps[:],
                        ld_sb[:, d * RANK:(d + 1) * RANK],
                        ct_sb[:, d * ROWS_PER_CORE:(d + 1) * ROWS_PER_CORE],
                        start=(d == 0), stop=(d == 7))
                dn_sb = smallp.tile([RANK, ROWS_PER_CORE], F32R, tag="dn")
                nc.vector.tensor_copy(dn_sb[:], dn_ps[:])
                for j in range(ROWS_PER_CORE // 128):
                    lo_sb = outp.tile([128, D], F32, tag="osb", name="lo_sb")
                    for g in range(2):
                        up_ps = op_ps.tile([128, 512], F32, tag="op",
                                           name="up_ps")
                        nc.tensor.matmul(
                            up_ps[:],
                            dn_sb[:, j * 128:(j + 1) * 128],
                            lu_sb[:, g * 512:(g + 1) * 512],
                            start=True, stop=True)
                        nc.vector.tensor_add(
                            lo_sb[:, g * 512:(g + 1) * 512], up_ps[:],
                            bo_bc[:, g * 512:(g + 1) * 512])
                    nc.sync.dma_start(out=lora_out[j * 128:(j + 1) * 128, :],
                                      in_=lo_sb[:])

            def emit_qkv(b):
                if b not in xt_tiles:
                    load_xt(b)
                xt = xt_tiles[b]

                qt_sb = qkvp.tile([HD, S], F32R, tag="qt", name=f"qt{b}")
                kt_sb = qkvp.tile([HD, S], F32R, tag="kt", name=f"kt{b}")
                vt_sb = qkvp.tile([HD, S], F32, tag="vt", bufs=1,
                                  name=f"vt{b}")
                v_sb = qkvp.tile([128, S], F32R, tag="v", name=f"v{b}")

                def proj_half(w_sb, dst, strip, h):
                    # contraction split in two PSUM groups per strip so the
                    # h=0 half can run while the second xT half still loads
                    ps2 = sc_ps.tile([128, 1024], F32, tag="sc", name="ps2")
                    ps = ps2[:, 0:512]
                    for dl in range(4):
                        d = h * 4 + dl
                        nc.tensor.matmul(
                            ps,
                            w_sb[:, d * HD:(d + 1) * HD],
                            xt[h][:, dl * S + strip * 512:
                                    dl * S + strip * 512 + 512],
                            start=(dl == 0), stop=(dl == 3),
                            skip_group_check=True)
                    sl = slice(strip * 512, (strip + 1) * 512)
                    if h == 0:
                        nc.vector.tensor_copy(dst[:, sl], ps)
                    else:
                        nc.vector.tensor_add(dst[:, sl], ps, dst[:, sl])

                for strip in range(NSTRIP):
                    proj_half(wv_sb, vt_sb, strip, 0)
                for strip in range(NSTRIP):
                    proj_half(wq_sb, qt_sb, strip, 0)
                for strip in range(NSTRIP):
                    proj_half(wk_sb, kt_sb, strip, 0)
                # second halves; V transposes interleave to keep PE dense
                for strip in range(NSTRIP):
                    proj_half(wv_sb, vt_sb, strip, 1)
                for strip in range(NSTRIP):
                    proj_half(wq_sb, qt_sb, strip, 1)
                    tq2 = sc_ps.tile([128, 1024], F32, tag="sc", name="tq2")
                    for i, kt in enumerate(range(4 * strip, 4 * strip + 4)):
                        nc.tensor.transpose(
                            tq2[:, i * 128:(i + 1) * 128],
                            vt_sb[:, kt * 128:(kt + 1) * 128], ident[:])
                    nc.vector.tensor_copy(
                        v_sb[:, strip * 512:(strip + 1) * 512],
                        tq2[:, 0:512])
                for strip in range(NSTRIP):
                    proj_half(wk_sb, kt_sb, strip, 1)
                return qt_sb, kt_sb, v_sb

            def emit_attention(b, qt_sb, kt_sb, v_sb):
                for strip in range(NSTRIP):
                    q_sl = slice(strip * 512, (strip + 1) * 512)
                    at_ps_t = at_ps.tile([HD, 512], F32, tag="at",
                                         name="at_ps_t")
                    sm_ps = sum_ps.tile([1, 512], F32, tag="sums",
                                        name="sm_ps")
                    for p in range(NKT // 2):
                        kt0 = 2 * p
                        scp = sc_ps.tile([128, 1024], F32, tag="sc",
                                         name="scp")
                        for i in range(2):
                            nc.tensor.matmul(
                                scp[:, i * 512:(i + 1) * 512],
                                kt_sb[:, (kt0 + i) * 128:(kt0 + i + 1) * 128],
                                qt_sb[:, q_sl],
                                start=True, stop=True,
                                skip_group_check=True)
                        es2 = esp.tile([128, 1024], F32R, tag="es",
                                       name="es2")
                        nc.scalar.activation(
                            es2[:], scp[:], mybir.ActivationFunctionType.Exp,
                            scale=float(INV_SQRT_HD))
                        for i in range(2):
                            kt = kt0 + i
                            nc.tensor.matmul(
                                at_ps_t[:],
                                v_sb[:, kt * 128:(kt + 1) * 128],
                                es2[:, i * 512:(i + 1) * 512],
                                start=(kt == 0), stop=(kt == NKT - 1),
                                skip_group_check=True)
                            nc.tensor.matmul(
                                sm_ps[:],
                                ones[:],
                                es2[:, i * 512:(i + 1) * 512],
                                start=(kt == 0), stop=(kt == NKT - 1),
                                skip_group_check=True)

                    # sums [1,512] -> SBUF row -> scatter to [128,4] columns
                    # -> 128-lane reciprocal (a [1,512] reciprocal would run
                    # serially on one DVE lane, ~3.3us)
                    row_sm = smallp.tile([1, 512], F32, tag="rowsm",
                                         name="row_sm")
                    nc.vector.tensor_copy(row_sm[:], sm_ps[:])
                    rcol_sb = smallp.tile([128, NQT], F32, tag="rcol",
                                          name="rcol_sb")
                    for j in range(NQT):
                        nc.sync.dma_start(
                            out=rcol_sb[:, j:j + 1],
                            in_=row_sm[0:1, j * 128:(j + 1) * 128])
                    rc_sb = smallp.tile([128, NQT], F32, tag="rc",
                                        name="rc_sb")
                    nc.vector.reciprocal(rc_sb[:], rcol_sb[:])

                    atn_sb = smallp.tile([HD, 512], F32R, tag="atn",
                                         name="atn_sb")
                    nc.vector.tensor_copy(atn_sb[:], at_ps_t[:])

                    # output projection + deferred softmax normalization
                    for j in range(NQT):
                        o_sb = outp.tile([128, D], F32, tag="osb", name="o_sb")
                        for g in range(2):
                            op = op_ps.tile([128, 512], F32, tag="op",
                                            name="op")
                            nc.tensor.matmul(
                                op[:],
                                atn_sb[:, j * 128:(j + 1) * 128],
                                wo_sb[:, g * 512:(g + 1) * 512],
                                start=True, stop=True)
                            nc.vector.tensor_scalar_mul(
                                o_sb[:, g * 512:(g + 1) * 512], op[:],
                                rc_sb[:, j:j + 1])
                        r0 = b * S + strip * 512 + j * 128
                        nc.sync.dma_start(out=out[r0:r0 + 128, :], in_=o_sb[:])

            qkv0 = emit_qkv(0)
            emit_lora()
            qkv1 = emit_qkv(1)
            emit_attention(0, *qkv0)
            emit_attention(1, *qkv1)

    nc.compile()
    _CACHE["nc"] = nc
    return nc


def _prep_in_maps(inputs):
    hidden = np.ascontiguousarray(inputs["hidden_states"], dtype=np.float32)
    control = np.ascontiguousarray(inputs["control_states"], dtype=np.float32)
    Wq = np.asarray(inputs["Wq"], dtype=np.float32)
    Wk = np.asarray(inputs["Wk"], dtype=np.float32)
    Wv = np.asarray(inputs["Wv"], dtype=np.float32)
    Wo = np.asarray(inputs["Wo"], dtype=np.float32)
    bo = np.asarray(inputs["bo"], dtype=np.float32)
    ld = np.asarray(inputs["lora_down"], dtype=np.float32)
    lu = np.asarray(inputs["lora_up"], dtype=np.float32)

    xT = np.ascontiguousarray(hidden.reshape(SG, D).T)
    cT_full = np.ascontiguousarray(control.reshape(SG, D).T)
    ldT = np.ascontiguousarray(ld.T)
    luT = np.ascontiguousarray(lu.T)
    bo_in = np.ascontiguousarray(bo.reshape(1, D))

    in_maps = []
    for c in range(N_CORES):
        hs = slice(c * HD, (c + 1) * HD)
        rs = slice(c * ROWS_PER_CORE, (c + 1) * ROWS_PER_CORE)
        in_maps.append({
            "xT": xT,
            "wqT": np.ascontiguousarray(Wq[hs, :].T),
            "wkT": np.ascontiguousarray(Wk[hs, :].T),
            "wvT": np.ascontiguousarray(Wv[hs, :].T),
            "woT": np.ascontiguousarray(Wo[:, hs].T),
            "cT": np.ascontiguousarray(cT_full[:, rs]),
            "ldT": ldT,
            "luT": luT,
            "bo": bo_in,
        })
    return in_maps


def _reduce_outputs(results):
    total = np.zeros((SG, D), dtype=np.float64)
    for c in range(N_CORES):
        total += results[c]["out"].astype(np.float64)
    total = total.astype(np.float32)
    for c in range(N_CORES):
        rs = slice(c * ROWS_PER_CORE, (c + 1) * ROWS_PER_CORE)
        total[rs] += results[c]["lora_out"]
    return total.reshape(B, S, D)


def kernel(**inputs):
    nc = build_program()
    in_maps = _prep_in_maps(inputs)
    res = run_bass_kernel_spmd(nc, in_maps, list(range(N_CORES)))
    return _reduce_outputs(res.results)



# revision 7
# speedup vs baseline: 1.0812x; 1.0812x over previous
"""Trainium2 Bass kernel for ControlLoRACrossAttnProcessor (v2).

Batch x head-group sharding over 8 NeuronCores: core c owns batch c//4
and heads 2*(c%4), 2*(c%4)+1.  Each core computes both heads' full
attention over its batch and a partial output projection (contraction
over its 256 Wo rows); the host sums 4 partials per batch.  The rank-4
LoRA path is sharded by global sequence rows (core c owns rows
c*512:(c+1)*512) and emitted at the START of the program so it doubles
as PE warm-up; the output bias rides the LoRA partial (added exactly
once per row).

All matmuls are bf16 (hosts casts inputs); PSUM accumulates fp32.
Attention uses the transposed-scores layout: scores^T tiles
[k=128, q=512] so softmax's exp rides ScalarE.  Softmax denominators
avoid the PE entirely: a DVE tree-add collapses the 16 exp tiles per
(head, strip) to [128, 512], GpSimd partition_all_reduce produces the
per-q sums broadcast over all partitions, DVE reciprocal + tensor_mul
normalize the PV accumulator before the (per-q linear) output
projection.
"""

import sys

for _p in ("/opt/trn_rl_repo", "/root/.axon_site"):
    if _p not in sys.path:
        sys.path.insert(0, _p)

import numpy as np
import ml_dtypes

import concourse.bass as bass  # noqa: E402
import concourse.mybir as mybir  # noqa: E402
import concourse.bass_isa as bass_isa  # noqa: E402
from concourse import bacc  # noqa: E402
from concourse.bass_utils import run_bass_kernel_spmd  # noqa: E402
from concourse.tile import TileContext  # noqa: E402
from concourse.masks import make_identity  # noqa: E402

dt = mybir.dt

B, S, D = 2, 2048, 1024
H = 8
HD = 128
RANK = 4
N_CORES = 8
SG = B * S
ROWS_PER_CORE = SG // N_CORES      # 512 lora rows per core
NSTRIP = S // 512                  # 4 query strips of 512
NKT = S // 128                     # 16 key tiles of 128
HEADS_PER_CORE = 2
INV_SQRT_HD = 1.0 / np.sqrt(np.float32(HD))

F32 = dt.float32
BF16 = dt.bfloat16

_CACHE = {}


def build_program():
    if "nc" in _CACHE:
        return _CACHE["nc"]

    nc = bacc.Bacc("TRN2", target_bir_lowering=False, debug=False,
                   num_devices=N_CORES)

    xT = nc.declare_dram_parameter("xT", [D, S], BF16, isOutput=False)
    wqT = nc.declare_dram_parameter("wqT", [D, 256], BF16, isOutput=False)
    wkT = nc.declare_dram_parameter("wkT", [D, 256], BF16, isOutput=False)
    wvT = nc.declare_dram_parameter("wvT", [D, 256], BF16, isOutput=False)
    woT = nc.declare_dram_parameter("woT", [256, D], BF16, isOutput=False)
    cT = nc.declare_dram_parameter("cT", [D, ROWS_PER_CORE], BF16,
                                   isOutput=False)
    ldT = nc.declare_dram_parameter("ldT", [D, RANK], BF16, isOutput=False)
    luT = nc.declare_dram_parameter("luT", [RANK, D], BF16, isOutput=False)
    bo = nc.declare_dram_parameter("bo", [1, D], F32, isOutput=False)
    out = nc.declare_dram_parameter("out", [S, D], BF16, isOutput=True)
    lora_out = nc.declare_dram_parameter("lora_out", [ROWS_PER_CORE, D],
                                         BF16, isOutput=True)

    with TileContext(nc) as tc:
        with tc.tile_pool(name="const", bufs=1) as constp, \
             tc.tile_pool(name="wts", bufs=1) as wts, \
             tc.tile_pool(name="xs", bufs=1) as xsp, \
             tc.tile_pool(name="qkv", bufs=1) as qkvp, \
             tc.tile_pool(name="stage", bufs=2) as stagep, \
             tc.tile_pool(name="es", bufs=10) as esp, \
             tc.tile_pool(name="sp", bufs=8) as spp, \
             tc.tile_pool(name="small", bufs=2) as smallp, \
             tc.tile_pool(name="outp", bufs=3) as outp, \
             tc.tile_pool(name="sc_ps", bufs=2, space="PSUM") as sc_ps, \
             tc.tile_pool(name="at_ps", bufs=2, space="PSUM") as at_ps, \
             tc.tile_pool(name="op_ps", bufs=2, space="PSUM") as op_ps:

            # ---- constants ----
            ident = constp.tile([128, 128], BF16, tag="ident")
            make_identity(nc, ident[:])
            dummy = constp.tile([128, 512], BF16, tag="dummy")
            nc.vector.memset(dummy[:], 0.0)

            # ---- DMA loads, ordered for earliest compute start ----
            ld_sb = wts.tile([128, 8 * RANK], BF16, tag="ld")
            lu_sb = wts.tile([RANK, D], BF16, tag="lu")
            bo_sb = wts.tile([1, D], F32, tag="bo")
            bo_bc = wts.tile([128, D], F32, tag="bobc")
            ct_sb = wts.tile([128, 8 * ROWS_PER_CORE], BF16, tag="ct")
            nc.sync.dma_start(out=ld_sb[:].rearrange("p (t m) -> p t m", t=8),
                              in_=ldT[:].rearrange("(t p) m -> p t m", p=128))
            nc.sync.dma_start(out=lu_sb[:], in_=luT[:])
            nc.sync.dma_start(out=bo_sb[:], in_=bo[:])
            nc.sync.dma_start(
                out=ct_sb[:].rearrange("p (t m) -> p t m", t=8),
                in_=cT[:].rearrange("(t p) m -> p t m", p=128))

            w_sb = {}   # (which, head) -> [128, 8*128] tile
            for h in range(HEADS_PER_CORE):
                for nm, src in (("q", wqT), ("k", wkT), ("v", wvT)):
                    t = wts.tile([128, 8 * HD], BF16, tag=f"w{nm}{h}",
                                 name=f"w{nm}{h}")
                    w_sb[(nm, h)] = t
                if h == 0:
                    for nm, src in (("q", wqT), ("k", wkT), ("v", wvT)):
                        nc.sync.dma_start(
                            out=w_sb[(nm, 0)][:].rearrange(
                                "p (t m) -> p t m", t=8),
                            in_=src[:, 0:HD].rearrange(
                                "(t p) m -> p t m", p=128))

            xs = []
            for s in range(NSTRIP):
                t = xsp.tile([128, 8 * 512], BF16, tag=f"xs{s}",
                             name=f"xs{s}")
                xs.append(t)
                for c in range(8):
                    nc.sync.dma_start(
                        out=t[:, c * 512:(c + 1) * 512],
                        in_=xT[c * 128:(c + 1) * 128, s * 512:(s + 1) * 512])

            for nm, src in (("q", wqT), ("k", wkT), ("v", wvT)):
                nc.sync.dma_start(
                    out=w_sb[(nm, 1)][:].rearrange("p (t m) -> p t m", t=8),
                    in_=src[:, HD:2 * HD].rearrange("(t p) m -> p t m", p=128))

            wo_sb = [wts.tile([HD, D], BF16, tag=f"wo{h}", name=f"wo{h}")
                     for h in range(HEADS_PER_CORE)]
            for h in range(HEADS_PER_CORE):
                nc.sync.dma_start(out=wo_sb[h][:],
                                  in_=woT[h * HD:(h + 1) * HD, :])

            # ---- PE warmup on the zero tile while first DMAs land ----
            for _wu in range(10):
                wu_ps = op_ps.tile([128, 512], F32, tag="op", name="wu_ps")
                nc.tensor.matmul(wu_ps[:], dummy[:, 0:128], dummy[:],
                                 start=True, stop=True)

            nc.gpsimd.partition_broadcast(bo_bc[:], bo_sb[:])

            # ---- LoRA path (also serves as further PE warmup) ----
            def emit_lora():
                dn_ps = op_ps.tile([128, ROWS_PER_CORE], F32, tag="op",
                                   name="dn_ps")
                for d in range(8):
                    nc.tensor.matmul(
                        dn_ps[0:RANK, :],
                        ld_sb[:, d * RANK:(d + 1) * RANK],
                        ct_sb[:, d * ROWS_PER_CORE:(d + 1) * ROWS_PER_CORE],
                        start=(d == 0), stop=(d == 7))
                dn_sb = smallp.tile([RANK, ROWS_PER_CORE], BF16, tag="dn",
                                    name="dn_sb")
                nc.scalar.copy(dn_sb[:], dn_ps[0:RANK, :])
                for j in range(ROWS_PER_CORE // 128):
                    lo_sb = outp.tile([128, D], BF16, tag="osb", name="lo_sb")
                    for g in range(2):
                        up_ps = op_ps.tile([128, 512], F32, tag="op",
                                           name="up_ps")
                        nc.tensor.matmul(
                            up_ps[:],
                            dn_sb[:, j * 128:(j + 1) * 128],
                            lu_sb[:, g * 512:(g + 1) * 512],
                            start=True, stop=True)
                        nc.vector.tensor_add(
                            lo_sb[:, g * 512:(g + 1) * 512], up_ps[:],
                            bo_bc[:, g * 512:(g + 1) * 512])
                    nc.sync.dma_start(out=lora_out[j * 128:(j + 1) * 128, :],
                                      in_=lo_sb[:])

            # ---- QKV projection for one (head, strip) ----
            qt = [qkvp.tile([HD, S], BF16, tag=f"qt{h}", name=f"qt{h}")
                  for h in range(HEADS_PER_CORE)]
            kt = [qkvp.tile([HD, S], BF16, tag=f"kt{h}", name=f"kt{h}")
                  for h in range(HEADS_PER_CORE)]
            v_sb = [qkvp.tile([128, S], BF16, tag=f"v{h}", name=f"v{h}")
                    for h in range(HEADS_PER_CORE)]

            def qkv_strip(h, s, ce):
                sl = slice(s * 512, (s + 1) * 512)

                def proj(nm):
                    ps = op_ps.tile([128, 512], F32, tag="op", name="pj_ps")
                    for d in range(8):
                        nc.tensor.matmul(
                            ps[:],
                            w_sb[(nm, h)][:, d * HD:(d + 1) * HD],
                            xs[s][:, d * 512:(d + 1) * 512],
                            start=(d == 0), stop=(d == 7),
                            skip_group_check=True)
                    return ps

                ps_v = proj("v")
                vt_stage = stagep.tile([HD, 512], BF16, tag="vst",
                                       name="vt_stage")
                ce(vt_stage[:], ps_v[:])
                tq = op_ps.tile([128, 512], BF16, tag="op", name="tq")
                for i in range(4):
                    nc.tensor.transpose(tq[:, i * 128:(i + 1) * 128],
                                        vt_stage[:, i * 128:(i + 1) * 128],
                                        ident[:])
                ce(v_sb[h][:, sl], tq[:])

                ps_q = proj("q")
                ce(qt[h][:, sl], ps_q[:])
                ps_k = proj("k")
                ce(kt[h][:, sl], ps_k[:])

            # ---- attention core for one (head, strip) ----
            atn = [qkvp.tile([HD, S], BF16, tag=f"atn{h}", name=f"atn{h}")
                   for h in range(HEADS_PER_CORE)]

            def attn_core(h, s):
                q_sl = slice(s * 512, (s + 1) * 512)
                at = at_ps.tile([HD, 512], F32, tag="at", name="at")
                tlev = []  # pair-sum tiles [128, 1024]
                for p in range(8):
                    scp = sc_ps.tile([128, 1024], F32, tag="sc", name="scp")
                    for i in range(2):
                        ktile = 2 * p + i
                        nc.tensor.matmul(
                            scp[:, i * 512:(i + 1) * 512],
                            kt[h][:, ktile * 128:(ktile + 1) * 128],
                            qt[h][:, q_sl],
                            start=True, stop=True, skip_group_check=True)
                    es_p = esp.tile([128, 1024], BF16, tag="es", name="es_p")
                    nc.scalar.activation(
                        es_p[:], scp[:], mybir.ActivationFunctionType.Exp,
                        scale=float(INV_SQRT_HD))
                    for i in range(2):
                        ktile = 2 * p + i
                        nc.tensor.matmul(
                            at[:],
                            v_sb[h][:, ktile * 128:(ktile + 1) * 128],
                            es_p[:, i * 512:(i + 1) * 512],
                            start=(ktile == 0), stop=(ktile == NKT - 1),
                            skip_group_check=True)
                    if p % 2 == 1:
                        t = spp.tile([128, 1024], BF16, tag="sp", name="tsum")
                        nc.vector.tensor_add(t[:], prev_es[:], es_p[:])
                        tlev.append(t)
                    prev_es = es_p
                # tree: 4 -> 2 -> 1, then fold halves
                nc.vector.tensor_add(tlev[0][:], tlev[0][:], tlev[1][:])
                nc.vector.tensor_add(tlev[2][:], tlev[2][:], tlev[3][:])
                nc.vector.tensor_add(tlev[0][:], tlev[0][:], tlev[2][:])
                spf = smallp.tile([128, 512], BF16, tag="spf", name="spf")
                nc.vector.tensor_add(spf[:], tlev[0][:, 0:512],
                                     tlev[0][:, 512:1024])
                den = smallp.tile([128, 512], F32, tag="den", name="den")
                nc.gpsimd.partition_all_reduce(
                    den[:], spf[:], channels=128,
                    reduce_op=bass_isa.ReduceOp.add)
                rc = smallp.tile([128, 512], F32, tag="rc", name="rc")
                nc.vector.reciprocal(rc[:], den[:])
                nc.vector.tensor_mul(atn[h][:, q_sl], at[:], rc[:])

            # ---- output projection for one strip (both heads) ----
            def out_proj(s):
                for j in range(4):
                    c_sl = slice(s * 512 + j * 128, s * 512 + (j + 1) * 128)
                    o_sb = outp.tile([128, D], BF16, tag="osb", name="o_sb")
                    for g in range(2):
                        ps = op_ps.tile([128, 512], F32, tag="op",
                                        name="opj_ps")
                        for h in range(HEADS_PER_CORE):
                            nc.tensor.matmul(
                                ps[:],
                                atn[h][:, c_sl],
                                wo_sb[h][:, g * 512:(g + 1) * 512],
                                start=(h == 0), stop=(h == 1),
                                skip_group_check=True)
                        nc.vector.tensor_copy(o_sb[:, g * 512:(g + 1) * 512],
                                              ps[:])
                    r0 = s * 512 + j * 128
                    nc.sync.dma_start(out=out[r0:r0 + 128, :], in_=o_sb[:])

            # ---- schedule ----
            emit_lora()
            for s in range(NSTRIP):
                qkv_strip(0, s, nc.scalar.copy)
            for s in range(NSTRIP):
                attn_core(0, s)
                qkv_strip(1, s, nc.vector.tensor_copy)
            for s in range(NSTRIP):
                attn_core(1, s)
                if s >= 2:
                    out_proj(s - 2)
            out_proj(2)
            out_proj(3)

    nc.compile()
    _CACHE["nc"] = nc
    return nc


def _prep_in_maps(inputs):
    bf = ml_dtypes.bfloat16
    hidden = np.asarray(inputs["hidden_states"], dtype=np.float32)
    control = np.asarray(inputs["control_states"], dtype=np.float32)
    Wq = np.asarray(inputs["Wq"], dtype=np.float32)
    Wk = np.asarray(inputs["Wk"], dtype=np.float32)
    Wv = np.asarray(inputs["Wv"], dtype=np.float32)
    Wo = np.asarray(inputs["Wo"], dtype=np.float32)
    bo_in = np.ascontiguousarray(
        np.asarray(inputs["bo"], dtype=np.float32).reshape(1, D))
    ldT = np.ascontiguousarray(
        np.asarray(inputs["lora_down"], dtype=np.float32).T.astype(bf))
    luT = np.ascontiguousarray(
        np.asarray(inputs["lora_up"], dtype=np.float32).T.astype(bf))

    xT_b = [np.ascontiguousarray(hidden[b].T.astype(bf)) for b in range(B)]
    cT_full = control.reshape(SG, D).T.astype(bf)

    in_maps = []
    for c in range(N_CORES):
        b = c // 4
        g = c % 4
        hs = slice(g * 256, (g + 1) * 256)
        rs = slice(c * ROWS_PER_CORE, (c + 1) * ROWS_PER_CORE)
        in_maps.append({
            "xT": xT_b[b],
            "wqT": np.ascontiguousarray(Wq[hs, :].T.astype(bf)),
            "wkT": np.ascontiguousarray(Wk[hs, :].T.astype(bf)),
            "wvT": np.ascontiguousarray(Wv[hs, :].T.astype(bf)),
            "woT": np.ascontiguousarray(Wo[:, hs].T.astype(bf)),
            "cT": np.ascontiguousarray(cT_full[:, rs]),
            "ldT": ldT,
            "luT": luT,
            "bo": bo_in,
        })
    return in_maps


def _reduce_outputs(results):
    total = np.zeros((B, S, D), dtype=np.float32)
    for c in range(N_CORES):
        b = c // 4
        total[b] += results[c]["out"].astype(np.float32)
    flat = total.reshape(SG, D)
    for c in range(N_CORES):
        rs = slice(c * ROWS_PER_CORE, (c + 1) * ROWS_PER_CORE)
        flat[rs] += results[c]["lora_out"].astype(np.float32)
    return flat.reshape(B, S, D)


def kernel(**inputs):
    nc = build_program()
    in_maps = _prep_in_maps(inputs)
    res = run_bass_kernel_spmd(nc, in_maps, list(range(N_CORES)))
    return _reduce_outputs(res.results)
